# revision 63
# baseline (speedup 1.0000x reference)
"""Multi-head causal attention on 8 Trainium2 cores (v3).

Sharding: 8 cores = 4 batches x 2 head-groups (8 heads each); host sums the
two head-group partials per batch (the "all-reduce") and pre-transposes +
pre-casts x/pos/W to bf16 per shard (pure layout/precision prep -- the
device matmuls consume bf16 anyway), halving input DMA bytes and removing
every on-device weight/x cast.  An identity-matmul spin at kernel start
warms the PE HAM clock gate (else the first ~3.4us of matmuls run at
1.2GHz instead of 2.4).

Per-core dataflow (bf16 matmul operands, fp32 PSUM):
  B(sb): x halves DMA over scalar+gpsimd queues into xTb; pos over sync
         straight into xqTb, added in place (every SBUF buffer is written
         by exactly one DMA queue -- completion order is only FIFO within
         a queue); QT/KT [ih-pair, chunk, seq] accumulate over 8 m-chunks;
         V laid out per CW=176 chunk as
         [valsA(64) | onesA | pad | onesB(@80) | valsB(112:176)] so the two
         head-halves of z land on disjoint, 32-aligned PSUM partition ranges:
           zps[:,0] = V[176c    :+128].T @ ex0 -> z_A rows 0:64,  Z_A row 64
           zps[:,1] = V[176c+48 :+128].T @ ex1 -> Z_B row 32, z_B rows 64:128
  C(c,qb): per key tile: scoresT [k, 2-head, q] via row-paired
         (tile_position) matmuls that run concurrently on the two 64-row PE
         groups, diagonal tiles column-trimmed; one ACT exp (scale=1/8)
         covers both heads; the causal staircase is a DVE multiply on the
         exp'd diagonal tile (the PE runs no mask matmuls); zps += V_kt.T@ex.
  norm:  whole zps tile DVE-drained to zsb (one free-dim-bound copy); per
         chunk pair the Z rows DMA to partition 0, one batched
         reciprocal_approx_fast per head, gpsimd partition-broadcast (A at
         base 0; B staged at base 0 and partition-shift-DMA'd to 64:128 --
         the broadcast ucode only writes from partition 0), then 2 DVE
         mults write both head-halves of zTf with no partition shuffling.
  D(qb): out[q, m] accumulates zTf.T @ woT over 4 chunks -> osb -> DMA out
         (sync queue).
Schedule: emission interleaves B(sb+1) load/proj quarter-chain units and
deferred D-wave units into C(qb)'s key-tile loop so the PE stays dense
while ACT chews the exps (ACT and PE are nearly balanced in attention
waves).  The last wave runs per-chunk norms; D(3) splits into a 3-chunk
partial accumulated into d3osb mid-wave plus a chunk-3 matmul + DVE add in
the tail; all of D(2) is held back to keep the PE warm while the final
norm chain resolves, whose 1/Z broadcast runs as two rank-1 PE matmuls
(bf16 1/Z via a scalar-engine copy) since the PE is idle there.
"""

import sys

if "/opt/trn_rl_repo" not in sys.path:
    sys.path.insert(0, "/opt/trn_rl_repo")

import numpy as np
import ml_dtypes

SEQ = 2048
DM = 1024
NH = 8           # heads per core
DH = 64
IH = NH * DH     # 512
MC = DM // 128   # 8 m-chunks
ST = SEQ // 128  # 16 seq tiles
NQB = SEQ // 512  # 4 query blocks
NC_CH = NH // 2  # 4 head-pair chunks
CW = 176         # V chunk stride: valsA 0:64, onesA 64, onesB 80, valsB 112:176
# Schraudolph-style exp for bf16 via the int16 bit pattern:
# bits16(exp(0.125*s)) ~= round(EXP_A*s + EXP_B), max rel err ~3.3%.
# Safe for unmasked (off-diagonal) score tiles (|s| < ~700).
EXP_A = 128 * 0.125 * 1.4426950408889634
EXP_B = 16250.5

_BUILT = None


def _build():
    import concourse.mybir as mybir
    import concourse.tile as tile
    from concourse import bacc
    from concourse.masks import make_identity

    dt = mybir.dt
    f32, bf16 = dt.float32, dt.bfloat16
    AF = mybir.ActivationFunctionType
    Alu = mybir.AluOpType

    nc = bacc.Bacc("TRN2", target_bir_lowering=False, debug=False)
    xT_d = nc.dram_tensor("xT_s", [DM, SEQ], bf16, kind="ExternalInput")
    posT_d = nc.dram_tensor("posT_s", [DM, SEQ], bf16, kind="ExternalInput")
    wqT_d = nc.dram_tensor("wqT_s", [DM, IH], bf16, kind="ExternalInput")
    wkT_d = nc.dram_tensor("wkT_s", [DM, IH], bf16, kind="ExternalInput")
    wvT_d = nc.dram_tensor("wvT_s", [DM, IH], bf16, kind="ExternalInput")
    woT_d = nc.dram_tensor("woT_s", [128, NC_CH, DM], bf16, kind="ExternalInput")
    out_d = nc.dram_tensor("out_s", [SEQ, DM], f32, kind="ExternalOutput")

    with tile.TileContext(nc) as tc:
        with tc.tile_pool(name="const", bufs=1) as cp, \
             tc.tile_pool(name="big", bufs=1) as bigp, \
             tc.tile_pool(name="wts", bufs=1) as wp, \
             tc.tile_pool(name="xblk", bufs=1) as xblk, \
             tc.tile_pool(name="expp", bufs=3) as expp, \
             tc.tile_pool(name="norm", bufs=1) as npl, \
             tc.tile_pool(name="outsb", bufs=2) as outsb, \
             tc.tile_pool(name="mm", bufs=2, space="PSUM") as mmp, \
             tc.tile_pool(name="sc", bufs=2, space="PSUM") as scp, \
             tc.tile_pool(name="zp", bufs=1, space="PSUM") as zpp:

            # ---------------- constants -------------------------------
            identb = cp.tile([128, 128], bf16)
            make_identity(nc, identb[:])
            # stair2[p, hh, c] = 1 if c >= p else 0 (causal keep-mask,
            # replicated for both heads so one DVE mult covers the pair)
            stair2 = cp.tile([128, 2, 128], bf16)
            nc.gpsimd.memset(stair2[:], 1.0)
            nc.gpsimd.affine_select(
                out=stair2[:], in_=stair2[:], compare_op=Alu.is_ge,
                fill=0.0, base=0, pattern=[[0, 2], [1, 128]],
                channel_multiplier=-1)
            ones_st = cp.tile([128, 1], f32)
            nc.gpsimd.memset(ones_st[:], 1.0)
            ones_row = cp.tile([1, 64], bf16)
            nc.gpsimd.memset(ones_row[:], 1.0)
            zero_st = cp.tile([128, 1], f32)
            nc.gpsimd.memset(zero_st[:], 0.0)

            # ---------------- persistent SBUF tensors -----------------
            QT = bigp.tile([128, NC_CH, SEQ], bf16)   # [pair-dim, chunk, seq]
            KT = bigp.tile([128, NC_CH, SEQ], bf16)
            V = bigp.tile([128, ST, NC_CH * CW], bf16)
            zTf = bigp.tile([128, NC_CH, SEQ], bf16)  # [pair-dim, chunk, q]
            # each weight tensor is split into two half-tiles so its DMAs
            # can ride both queues while keeping one-buffer-one-queue (DMA
            # completion ordering is only FIFO within a queue)
            wqT = [wp.tile([128, 4, IH], bf16, name=f"wqT{h}")
                   for h in range(2)]
            wkT = [wp.tile([128, 4, IH], bf16, name=f"wkT{h}")
                   for h in range(2)]
            wvT = [wp.tile([128, 4, IH], bf16, name=f"wvT{h}")
                   for h in range(2)]
            woT = wp.tile([128, NC_CH, DM], bf16)     # [pair-dim, chunk, m]
            zsb = bigp.tile([128, 2, NC_CH, 512], f32)  # norm staging per qb

            # ---------------- PE clock warmup -------------------------
            wu = mmp.tile([128, 512], f32, tag="mm", name="warmup")
            for _ in range(32):
                nc.tensor.matmul(wu[:, 0:128], identb[:], identb[:],
                                 start=True, stop=True)

            # V ones columns (softmax normalizer rides the zps matmul) and
            # zero fill for the pad region between the head-halves.
            nc.vector.tensor_copy(
                V[:].rearrange("p s (c w) -> p s c w", c=NC_CH)[
                    :, :, :, 64:112],
                zero_st[:, 0:1].to_broadcast([128, ST, NC_CH, 48]))
            nc.vector.tensor_copy(
                V[:].rearrange("p s (c w) -> p s c w", c=NC_CH)[
                    :, :, :, 64:65],
                ones_st[:, 0:1].to_broadcast([128, ST, NC_CH, 1]))
            nc.vector.tensor_copy(
                V[:].rearrange("p s (c w) -> p s c w", c=NC_CH)[
                    :, :, :, 80:81],
                ones_st[:, 0:1].to_broadcast([128, ST, NC_CH, 1]))

            # ---------------- weight loads ----------------------------
            def w_chunk_units(w_d, wT, engs):
                units = []
                for mc in range(MC):
                    def u(mc=mc):
                        engs[mc % 2].dma_start(
                            wT[mc % 2][:, mc // 2, :],
                            w_d.ap()[mc * 128:(mc + 1) * 128, :])
                    units.append(u)
                return units

            def wo_units():
                units = []
                for c in range(NC_CH):
                    def u(c=c):
                        nc.gpsimd.dma_start(woT[:, c, :], woT_d.ap()[:, c, :])
                    units.append(u)
                return units

            # ---------------- work-unit machinery ---------------------
            def b_load_units(sb):
                xqTb = xblk.tile([128, MC, 512], bf16, tag=f"xq{sb % 2}",
                                 name=f"xqTb{sb}")
                # x halves ride two queues (scalar + gpsimd); pos rides sync
                # straight into xqTb, added in place
                xTb = [xblk.tile([128, 4, 512], bf16, tag=f"xt{sb % 2}{h}",
                                 name=f"xTb{sb}{h}") for h in range(2)]
                xengs = [nc.scalar, nc.gpsimd]
                units = []
                for mc in range(MC):
                    def u(mc=mc, xqTb=xqTb, xTb=xTb):
                        xengs[mc % 2].dma_start(
                            xTb[mc % 2][:, mc // 2, :],
                            xT_d.ap()[mc * 128:(mc + 1) * 128,
                                      sb * 512:(sb + 1) * 512])
                        nc.sync.dma_start(
                            xqTb[:, mc, :],
                            posT_d.ap()[mc * 128:(mc + 1) * 128,
                                        sb * 512:(sb + 1) * 512])
                        nc.vector.tensor_add(xqTb[:, mc, :], xqTb[:, mc, :],
                                             xTb[mc % 2][:, mc // 2, :])
                    units.append(u)
                return (xqTb, xTb), units

            def qk_proj_units(sb, blks, wT, dstT):
                xqTb, _ = blks
                units = []
                for c in range(NC_CH):
                    hold = {}
                    def mk(m0, m1, c=c, hold=hold):
                        def u():
                            if m0 == 0:
                                hold["ps"] = mmp.tile([128, 512], f32,
                                                      tag="mm", name="ps_qk")
                            ps = hold["ps"]
                            for mc in range(m0, m1):
                                nc.tensor.matmul(
                                    ps[:],
                                    wT[mc % 2][:, mc // 2,
                                               c * 128:(c + 1) * 128],
                                    xqTb[:, mc, :],
                                    start=(mc == 0), stop=(mc == MC - 1))
                            if m1 == MC:
                                nc.vector.tensor_copy(
                                    dstT[:, c, sb * 512:(sb + 1) * 512],
                                    ps[:])
                        return u
                    units += [mk(0, 2), mk(2, 4), mk(4, 6), mk(6, 8)]
                return units

            def v_proj_units(sb, blks):
                _, xTb = blks
                units = []
                for stl in range(4):
                    st = sb * 4 + stl
                    hold = {}
                    def uA(stl=stl, hold=hold):
                        ps = mmp.tile([128, 512], f32, tag="mm",
                                      name="ps_v")
                        hold["ps"] = ps
                        for mc in range(4):
                            nc.tensor.matmul(
                                ps[:],
                                xTb[mc % 2][:, mc // 2,
                                            stl * 128:(stl + 1) * 128],
                                wvT[mc % 2][:, mc // 2, :],
                                start=(mc == 0), stop=False)
                    def uB(st=st, stl=stl, hold=hold):
                        ps = hold["ps"]
                        for mc in range(4, MC):
                            nc.tensor.matmul(
                                ps[:],
                                xTb[mc % 2][:, mc // 2,
                                            stl * 128:(stl + 1) * 128],
                                wvT[mc % 2][:, mc // 2, :],
                                start=False, stop=(mc == MC - 1))
                        # heads land even -> cols 0:64, odd -> cols 112:176
                        # of their CW-chunk (two copies: strides differ)
                        vv = V[:, st, :].rearrange("p (c w) -> p c w",
                                                   c=NC_CH)
                        pp = ps[:].rearrange("p (c g h) -> p c g h",
                                             c=NC_CH, g=2)
                        nc.vector.tensor_copy(vv[:, :, 0:64], pp[:, :, 0])
                        nc.vector.tensor_copy(vv[:, :, 112:176],
                                              pp[:, :, 1])
                    units += [uA, uB]
                return units

            def b_proj_units(sb, blks):
                return (qk_proj_units(sb, blks, wqT, QT)
                        + qk_proj_units(sb, blks, wkT, KT)
                        + v_proj_units(sb, blks))

            def d_units(qb):
                units = []
                for qtl in range(4):
                    qt = qb * 4 + qtl
                    osb = outsb.tile([128, DM], f32, tag="osb",
                                     name=f"osb{qt}")
                    for mb in range(2):
                        hold = {}
                        def uA(qt=qt, mb=mb, hold=hold):
                            po = mmp.tile([128, 512], f32, tag="mm",
                                          name="po")
                            hold["po"] = po
                            for c in range(2):
                                nc.tensor.matmul(
                                    po[:],
                                    zTf[:, c, qt * 128:(qt + 1) * 128],
                                    woT[:, c, mb * 512:(mb + 1) * 512],
                                    start=(c == 0), stop=False)
                        def uB(qt=qt, mb=mb, osb=osb, hold=hold):
                            po = hold["po"]
                            for c in range(2, NC_CH):
                                nc.tensor.matmul(
                                    po[:],
                                    zTf[:, c, qt * 128:(qt + 1) * 128],
                                    woT[:, c, mb * 512:(mb + 1) * 512],
                                    start=False, stop=(c == NC_CH - 1))
                            nc.vector.tensor_copy(
                                osb[:, mb * 512:(mb + 1) * 512], po[:])
                            nc.sync.dma_start(
                                out_d.ap()[qt * 128:(qt + 1) * 128,
                                           mb * 512:(mb + 1) * 512],
                                osb[:, mb * 512:(mb + 1) * 512])
                        units += [uA, uB]
                return units

            # D(3) split: chunks 0-2 accumulate mid-wave into d3osb (fp32),
            # so the tail only runs the chunk-3 matmul + add + out DMA.
            d3osb = bigp.tile([128, 4, DM], f32)

            def d3_units():
                a_units, b_units = [], []
                for qtl in range(4):
                    qt = 12 + qtl
                    for mb in range(2):
                        def uA(qt=qt, qtl=qtl, mb=mb):
                            po = mmp.tile([128, 512], f32, tag="mm",
                                          name="po3a")
                            for c in range(3):
                                nc.tensor.matmul(
                                    po[:],
                                    zTf[:, c, qt * 128:(qt + 1) * 128],
                                    woT[:, c, mb * 512:(mb + 1) * 512],
                                    start=(c == 0), stop=(c == 2))
                            nc.vector.tensor_copy(
                                d3osb[:, qtl, mb * 512:(mb + 1) * 512],
                                po[:])
                        def uB(qt=qt, qtl=qtl, mb=mb):
                            po = mmp.tile([128, 512], f32, tag="mm",
                                          name="po3b")
                            nc.tensor.matmul(
                                po[:],
                                zTf[:, 3, qt * 128:(qt + 1) * 128],
                                woT[:, 3, mb * 512:(mb + 1) * 512],
                                start=True, stop=True)
                            nc.vector.tensor_add(
                                d3osb[:, qtl, mb * 512:(mb + 1) * 512],
                                d3osb[:, qtl, mb * 512:(mb + 1) * 512],
                                po[:])
                            nc.scalar.dma_start(
                                out_d.ap()[qt * 128:(qt + 1) * 128,
                                           mb * 512:(mb + 1) * 512],
                                d3osb[:, qtl, mb * 512:(mb + 1) * 512])
                        a_units.append(uA)
                        b_units.append(uB)
                return a_units, b_units

            def emit_c(c, qb, zps):
                nkt = 4 * qb + 4
                for kt in range(nkt):
                    j = kt - 4 * qb
                    diag = j >= 0
                    off = 128 * j if diag else 0
                    sc = scp.tile([128, 2, 512], f32, tag="sc", name="sc")
                    for hh in range(2):
                        r0 = hh * 64
                        nc.tensor.matmul(
                            sc[:, hh, off:512],
                            KT[r0:r0 + 64, c, kt * 128:(kt + 1) * 128],
                            QT[r0:r0 + 64, c,
                               qb * 512 + off:(qb + 1) * 512],
                            start=True, stop=True,
                            tile_position=(r0, 0))
                    ex = expp.tile([128, 2, 512], bf16, tag="ex",
                                   name="ex")
                    if qb == 3 and not diag and kt % 5 == 2:
                        # the last wave is ACT-bound while its DVE is mostly
                        # idle: offload a few exp tiles via the Schraudolph
                        # bit-trick (one DVE op; scores here are unmasked
                        # and bounded so no clamp is needed)
                        nc.vector.tensor_scalar(
                            ex[:, :, off:512].bitcast(dt.int16),
                            sc[:, :, off:512], EXP_A, EXP_B,
                            Alu.mult, Alu.add)
                    else:
                        nc.scalar.activation(ex[:, :, off:512],
                                             sc[:, :, off:512],
                                             AF.Exp, scale=0.125)
                    if diag:
                        # zero the above-diagonal exp'd entries (DVE, off-PE)
                        nc.vector.tensor_mul(
                            ex[:, :, off:off + 128],
                            ex[:, :, off:off + 128], stair2[:])
                    for hh in range(2):
                        nc.tensor.matmul(
                            zps[:, hh, off:512],
                            V[:, kt, CW * c + 48 * hh:CW * c + 48 * hh + 128],
                            ex[:, hh, off:512],
                            start=(kt == 0), stop=(kt == nkt - 1))
                    yield

            def drain_z(c, zps):
                # single full-tile drain (z_A rows 0:64 + Z_A row 64 in the
                # hh=0 slice; Z_B row 32 + z_B rows 64:128 in hh=1); one op
                # since DVE cost is free-dim-driven, not partition-driven
                nc.vector.tensor_copy(zsb[:, :, c, :], zps[:, :, :])

            def emit_norm(qb, c0, n, dq=None, pe_bcast=False, zps=None):
                dq = dq or nc.sync
                cs = slice(c0, c0 + n)
                # Z rows sit at partitions 64 (head A) / 32 (head B); DMA
                # them to partition 0 (two queues, in parallel) so recip
                # reads from base 0.  On the tail path, read them straight
                # from PSUM so the chain doesn't wait for the full drain.
                zr0 = npl.tile([1, 2, 512], f32, tag="zr0", name="zr0")
                zr1 = npl.tile([1, 2, 512], f32, tag="zr1", name="zr1")
                dq.dma_start(zr0[:, 0:n, :], zsb[64:65, 0, cs, :])
                dq.dma_start(zr1[:, 0:n, :], zsb[32:33, 1, cs, :])
                ri0 = npl.tile([1, 2, 512], f32, tag="ri0", name="ri0")
                ri1 = npl.tile([1, 2, 512], f32, tag="ri1", name="ri1")
                nc.vector.reciprocal_approx_fast(out=ri0[:, 0:n, :],
                                                 in_=zr0[:, 0:n, :])
                nc.vector.reciprocal_approx_fast(out=ri1[:, 0:n, :],
                                                 in_=zr1[:, 0:n, :])
                if pe_bcast:
                    # PE is idle at the tail: broadcast 1/Z across the 64
                    # partitions with two tiny rank-1 matmuls instead of
                    # gpsimd broadcast + partition-shift DMA
                    assert n == 1
                    # bridge 1/Z to bf16 on the (tail-idle) scalar engine
                    rib = npl.tile([1, 2, 512], bf16, tag="rib", name="rib")
                    nc.scalar.copy(rib[:, 0, :], ri0[:, 0, :])
                    nc.scalar.copy(rib[:, 1, :], ri1[:, 0, :])
                    bcp = scp.tile([128, 2, 512], f32, tag="sc", name="bcp")
                    nc.tensor.matmul(bcp[0:64, 0, :], ones_row[:],
                                     rib[0:1, 0, :], start=True, stop=True)
                    nc.tensor.matmul(bcp[64:128, 1, :], ones_row[:],
                                     rib[0:1, 1, :], start=True, stop=True,
                                     tile_position=(0, 64))
                    nc.vector.tensor_mul(
                        zTf[0:64, cs, qb * 512:(qb + 1) * 512],
                        zsb[0:64, 0, cs, :], bcp[0:64, 0:1, :])
                    nc.vector.tensor_mul(
                        zTf[64:128, cs, qb * 512:(qb + 1) * 512],
                        zsb[64:128, 1, cs, :], bcp[64:128, 1:2, :])
                    return
                bcA = npl.tile([64, 2, 512], f32, tag="bcA", name="bcA")
                bcB = npl.tile([128, 2, 512], f32, tag="bcB", name="bcB")
                nc.gpsimd.partition_broadcast(bcA[:, 0:n, :],
                                              ri0[:, 0:n, :], channels=64)
                # stage the B broadcast in bcB's (unused) lower half, then
                # partition-shift it up to 64:128 by DMA
                nc.gpsimd.partition_broadcast(bcB[0:64, 0:n, :],
                                              ri1[:, 0:n, :], channels=64)
                dq.dma_start(bcB[64:128, 0:n, :], bcB[0:64, 0:n, :])
                nc.vector.tensor_mul(
                    zTf[0:64, cs, qb * 512:(qb + 1) * 512],
                    zsb[0:64, 0, cs, :], bcA[:, 0:n, :])
                nc.vector.tensor_mul(
                    zTf[64:128, cs, qb * 512:(qb + 1) * 512],
                    zsb[64:128, 1, cs, :], bcB[64:128, 0:n, :])

            # ---------------- main schedule ---------------------------
            def pipe2(units):
                """Reorder [uA0,uB0,uA1,uB1,...] -> 2-deep software
                pipeline [uA0,uA1,uB0,uA2,uB1,...] so a uB stalled on its
                second DMA half doesn't head-of-line-block a ready uA
                (the PE executes its queue in order; mmp has 2 buffers).
                """
                ua, ub = units[0::2], units[1::2]
                out = [ua[0]]
                for i in range(1, len(ua)):
                    out += [ua[i], ub[i - 1]]
                out.append(ub[-1])
                return out

            blks = {}
            blks[0], lu0 = b_load_units(0)
            wq_u = w_chunk_units(wqT_d, wqT, [nc.sync, nc.scalar])
            for a, b_ in zip(wq_u, lu0):
                a()
                b_()
            for u in w_chunk_units(wkT_d, wkT, [nc.sync, nc.scalar]):
                u()
            for u in pipe2(qk_proj_units(0, blks[0], wqT, QT)):
                u()
            for u in w_chunk_units(wvT_d, wvT, [nc.gpsimd, nc.gpsimd]):
                u()
            for u in pipe2(qk_proj_units(0, blks[0], wkT, KT)):
                u()
            for u in pipe2(v_proj_units(0, blks[0])):
                u()

            # w0: wo+B(1); w1: B(2)+D(0); w2: B(3); w3: D(1)+D(2) through
            # chunks 0-2 then D(3)-first-parts through chunk 3; tail: only
            # the chunk-3 D(3) partials.
            for qb in range(NQB):
                units = []
                if qb == 0:
                    units += wo_units()
                if qb + 1 < NQB:
                    blks[qb + 1], lu = b_load_units(qb + 1)
                    units += lu
                    units += b_proj_units(qb + 1, blks[qb + 1])
                hold_back = []
                if qb == 3:
                    # D(0)/D(1) land in wave 3's ACT-bound slack (the other
                    # waves are PE-oversubscribed by their pinned B-proj
                    # work); D(2) is held back for the tail, keeping the PE
                    # warm while the final norm chain resolves
                    units += d_units(0) + d_units(1)
                    hold_back = d_units(2)
                nkt = 4 * qb + 4
                last = qb == NQB - 1
                # last wave: pace `units` through chunks 0-2 (per-chunk
                # norms), d3a through chunk 3; other waves: pace across all
                # four chunks with norms per chunk pair
                pace_kts = (3 if last else NC_CH) * nkt
                done = 0
                emitted = 0
                for c in range(NC_CH):
                    in_d3 = last and c == NC_CH - 1
                    if in_d3:
                        cur, emitted, pace_kts, done = d3a, 0, nkt, 0
                    else:
                        cur = units
                    zps = zpp.tile([128, 2, 512], f32, tag="z", name="zps")
                    kt_in_c = 0
                    for _ in emit_c(c, qb, zps):
                        done += 1
                        kt_in_c += 1
                        if kt_in_c >= nkt:
                            break  # drain first; catch up after
                        target = min(len(cur),
                                     (len(cur) * done * 4) // (3 * pace_kts))
                        while emitted < target:
                            cur[emitted]()
                            emitted += 1
                    drain_z(c, zps)
                    if last:
                        emit_norm(qb, c, 1,
                                  dq=nc.scalar if c == NC_CH - 1 else None,
                                  pe_bcast=(c == NC_CH - 1), zps=zps)
                        if c == NC_CH - 2:
                            while emitted < len(units):
                                units[emitted]()
                                emitted += 1
                            d3a, d3b = d3_units()
                    elif c % 2 == 1:
                        emit_norm(qb, 2 * (c // 2), 2)
                    target = min(len(cur),
                                 (len(cur) * done * 4) // (3 * pace_kts))
                    while emitted < target:
                        cur[emitted]()
                        emitted += 1
                while emitted < len(cur):
                    cur[emitted]()
                    emitted += 1
            for u in hold_back:
                u()
            for u in d3b:
                u()

    nc.compile()
    return nc


def _get_nc():
    global _BUILT
    if _BUILT is None:
        _BUILT = _build()
    return _BUILT


def _prep_core(x_b, pos_b, wq_g, wk_g, wv_g, wo_g):
    bf = ml_dtypes.bfloat16
    woT = np.empty((128, NC_CH, DM), dtype=np.float32)
    for c in range(NC_CH):
        for hh in range(2):
            woT[hh * 64:(hh + 1) * 64, c, :] = wo_g[2 * c + hh].T
    return {
        "xT_s": np.ascontiguousarray(x_b.T).astype(bf),
        "posT_s": np.ascontiguousarray(pos_b.T).astype(bf),
        "wqT_s": np.ascontiguousarray(wq_g.reshape(IH, DM).T).astype(bf),
        "wkT_s": np.ascontiguousarray(wk_g.reshape(IH, DM).T).astype(bf),
        "wvT_s": np.ascontiguousarray(wv_g.reshape(IH, DM).T).astype(bf),
        "woT_s": woT.astype(bf),
    }


def run(inputs, trace=False):
    from concourse import bass_utils

    nc = _get_nc()
    x = np.asarray(inputs["x"], dtype=np.float32)
    pos = np.asarray(inputs["pos_embed"], dtype=np.float32)
    wq, wk, wv, wo = (np.asarray(inputs[k], dtype=np.float32)
                      for k in ("W_Q", "W_K", "W_V", "W_O"))
    in_maps = []
    for core in range(8):
        b, g = core // 2, core % 2
        hs = slice(g * NH, (g + 1) * NH)
        in_maps.append(_prep_core(x[b], pos[b], wq[hs], wk[hs], wv[hs],
                                  wo[hs]))
    res = bass_utils.run_bass_kernel_spmd(
        nc, in_maps, core_ids=list(range(8)), trace=trace)
    out = np.empty((4, SEQ, DM), dtype=np.float32)
    for b in range(4):
        out[b] = res.results[2 * b]["out_s"] + res.results[2 * b + 1]["out_s"]
    return out, res.exec_time_ns


def kernel(**inputs):
    try:
        out, _ = run(inputs, trace=False)
    except Exception:
        # transient NRT/device errors have been observed on this part; one
        # retry on the already-compiled module is cheap insurance
        out, _ = run(inputs, trace=False)
    return out


# revision 64
# speedup vs baseline: 1.0335x; 1.0335x over previous
"""Multi-head causal attention on 8 Trainium2 cores (v3).

Sharding: 8 cores = 4 batches x 2 head-groups (8 heads each); host sums the
two head-group partials per batch (the "all-reduce") and pre-transposes +
pre-casts x/pos/W to bf16 per shard (pure layout/precision prep -- the
device matmuls consume bf16 anyway), halving input DMA bytes and removing
every on-device weight/x cast.  An identity-matmul spin at kernel start
warms the PE HAM clock gate (else the first ~3.4us of matmuls run at
1.2GHz instead of 2.4).

Per-core dataflow (bf16 matmul operands, fp32 PSUM):
  B(sb): x halves DMA over scalar+gpsimd queues into xTb; pos over sync
         straight into xqTb, added in place (every SBUF buffer is written
         by exactly one DMA queue -- completion order is only FIFO within
         a queue); QT/KT [ih-pair, chunk, seq] accumulate over 8 m-chunks;
         V laid out per CW=176 chunk as
         [valsA(64) | onesA | pad | onesB(@80) | valsB(112:176)] so the two
         head-halves of z land on disjoint, 32-aligned PSUM partition ranges:
           zps[:,0] = V[176c    :+128].T @ ex0 -> z_A rows 0:64,  Z_A row 64
           zps[:,1] = V[176c+48 :+128].T @ ex1 -> Z_B row 32, z_B rows 64:128
  C(c,qb): per key tile: scoresT [k, 2-head, q] via row-paired
         (tile_position) matmuls that run concurrently on the two 64-row PE
         groups, diagonal tiles column-trimmed; one ACT exp (scale=1/8)
         covers both heads; the causal staircase is a DVE multiply on the
         exp'd diagonal tile (the PE runs no mask matmuls); zps += V_kt.T@ex.
  norm:  whole zps tile DVE-drained to zsb (one free-dim-bound copy); per
         chunk pair the Z rows DMA to partition 0, one batched
         reciprocal_approx_fast per head, gpsimd partition-broadcast (A at
         base 0; B staged at base 0 and partition-shift-DMA'd to 64:128 --
         the broadcast ucode only writes from partition 0), then 2 DVE
         mults write both head-halves of zTf with no partition shuffling.
  D(qb): out[q, m] accumulates zTf.T @ woT over 4 chunks -> osb -> DMA out
         (sync queue).
Schedule: emission interleaves B(sb+1) load/proj quarter-chain units and
deferred D-wave units into C(qb)'s key-tile loop so the PE stays dense
while ACT chews the exps (ACT and PE are nearly balanced in attention
waves).  The last wave runs per-chunk norms; D(3) splits into a 3-chunk
partial accumulated into d3osb mid-wave plus a chunk-3 matmul + DVE add in
the tail; all of D(2) is held back to keep the PE warm while the final
norm chain resolves, whose 1/Z broadcast runs as two rank-1 PE matmuls
(bf16 1/Z via a scalar-engine copy) since the PE is idle there.
"""

import sys

if "/opt/trn_rl_repo" not in sys.path:
    sys.path.insert(0, "/opt/trn_rl_repo")

import numpy as np
import ml_dtypes

SEQ = 2048
DM = 1024
NH = 8           # heads per core
DH = 64
IH = NH * DH     # 512
MC = DM // 128   # 8 m-chunks
ST = SEQ // 128  # 16 seq tiles
NQB = SEQ // 512  # 4 query blocks
NC_CH = NH // 2  # 4 head-pair chunks
CW = 176         # V chunk stride: valsA 0:64, onesA 64, onesB 80, valsB 112:176

_BUILT = None


def _build():
    import concourse.mybir as mybir
    import concourse.tile as tile
    from concourse import bacc
    from concourse.masks import make_identity

    dt = mybir.dt
    f32, bf16 = dt.float32, dt.bfloat16
    AF = mybir.ActivationFunctionType
    Alu = mybir.AluOpType

    nc = bacc.Bacc("TRN2", target_bir_lowering=False, debug=False)
    xT_d = nc.dram_tensor("xT_s", [DM, SEQ], bf16, kind="ExternalInput")
    posT_d = nc.dram_tensor("posT_s", [DM, SEQ], bf16, kind="ExternalInput")
    wqT_d = nc.dram_tensor("wqT_s", [DM, IH], bf16, kind="ExternalInput")
    wkT_d = nc.dram_tensor("wkT_s", [DM, IH], bf16, kind="ExternalInput")
    wvT_d = nc.dram_tensor("wvT_s", [DM, IH], bf16, kind="ExternalInput")
    woT_d = nc.dram_tensor("woT_s", [128, NC_CH, DM], bf16, kind="ExternalInput")
    out_d = nc.dram_tensor("out_s", [SEQ, DM], f32, kind="ExternalOutput")

    with tile.TileContext(nc) as tc:
        with tc.tile_pool(name="const", bufs=1) as cp, \
             tc.tile_pool(name="big", bufs=1) as bigp, \
             tc.tile_pool(name="wts", bufs=1) as wp, \
             tc.tile_pool(name="xblk", bufs=1) as xblk, \
             tc.tile_pool(name="expp", bufs=3) as expp, \
             tc.tile_pool(name="norm", bufs=1) as npl, \
             tc.tile_pool(name="outsb", bufs=2) as outsb, \
             tc.tile_pool(name="mm", bufs=2, space="PSUM") as mmp, \
             tc.tile_pool(name="sc", bufs=2, space="PSUM") as scp, \
             tc.tile_pool(name="zp", bufs=1, space="PSUM") as zpp:

            # ---------------- constants -------------------------------
            identb = cp.tile([128, 128], bf16)
            make_identity(nc, identb[:])
            # stair2[p, hh, c] = 1 if c >= p else 0 (causal keep-mask,
            # replicated for both heads so one DVE mult covers the pair)
            stair2 = cp.tile([128, 2, 128], bf16)
            nc.gpsimd.memset(stair2[:], 1.0)
            nc.gpsimd.affine_select(
                out=stair2[:], in_=stair2[:], compare_op=Alu.is_ge,
                fill=0.0, base=0, pattern=[[0, 2], [1, 128]],
                channel_multiplier=-1)
            ones_st = cp.tile([128, 1], f32)
            nc.gpsimd.memset(ones_st[:], 1.0)
            ones_row = cp.tile([1, 64], bf16)
            nc.gpsimd.memset(ones_row[:], 1.0)
            zero_st = cp.tile([128, 1], f32)
            nc.gpsimd.memset(zero_st[:], 0.0)

            # ---------------- persistent SBUF tensors -----------------
            QT = bigp.tile([128, NC_CH, SEQ], bf16)   # [pair-dim, chunk, seq]
            KT = bigp.tile([128, NC_CH, SEQ], bf16)
            V = bigp.tile([128, ST, NC_CH * CW], bf16)
            zTf = bigp.tile([128, NC_CH, SEQ], bf16)  # [pair-dim, chunk, q]
            # each weight tensor is split into two half-tiles so its DMAs
            # can ride both queues while keeping one-buffer-one-queue (DMA
            # completion ordering is only FIFO within a queue)
            wqT = [wp.tile([128, 4, IH], bf16, name=f"wqT{h}")
                   for h in range(2)]
            wkT = [wp.tile([128, 4, IH], bf16, name=f"wkT{h}")
                   for h in range(2)]
            wvT = [wp.tile([128, 4, IH], bf16, name=f"wvT{h}")
                   for h in range(2)]
            woT = wp.tile([128, NC_CH, DM], bf16)     # [pair-dim, chunk, m]
            zsb = bigp.tile([128, 2, NC_CH, 512], f32)  # norm staging per qb

            # ---------------- PE clock warmup -------------------------
            wu = mmp.tile([128, 512], f32, tag="mm", name="warmup")
            for _ in range(32):
                nc.tensor.matmul(wu[:, 0:128], identb[:], identb[:],
                                 start=True, stop=True)

            # V ones columns (softmax normalizer rides the zps matmul) and
            # zero fill for the pad region between the head-halves.
            nc.vector.tensor_copy(
                V[:].rearrange("p s (c w) -> p s c w", c=NC_CH)[
                    :, :, :, 64:112],
                zero_st[:, 0:1].to_broadcast([128, ST, NC_CH, 48]))
            nc.vector.tensor_copy(
                V[:].rearrange("p s (c w) -> p s c w", c=NC_CH)[
                    :, :, :, 64:65],
                ones_st[:, 0:1].to_broadcast([128, ST, NC_CH, 1]))
            nc.vector.tensor_copy(
                V[:].rearrange("p s (c w) -> p s c w", c=NC_CH)[
                    :, :, :, 80:81],
                ones_st[:, 0:1].to_broadcast([128, ST, NC_CH, 1]))

            # ---------------- weight loads ----------------------------
            def w_chunk_units(w_d, wT, engs):
                units = []
                for mc in range(MC):
                    def u(mc=mc):
                        engs[mc % 2].dma_start(
                            wT[mc % 2][:, mc // 2, :],
                            w_d.ap()[mc * 128:(mc + 1) * 128, :])
                    units.append(u)
                return units

            def wo_units():
                units = []
                for c in range(NC_CH):
                    def u(c=c):
                        nc.gpsimd.dma_start(woT[:, c, :], woT_d.ap()[:, c, :])
                    units.append(u)
                return units

            # ---------------- work-unit machinery ---------------------
            def b_load_units(sb):
                xqTb = xblk.tile([128, MC, 512], bf16, tag=f"xq{sb % 2}",
                                 name=f"xqTb{sb}")
                # x halves ride two queues (scalar + gpsimd); pos rides sync
                # straight into xqTb, added in place
                xTb = [xblk.tile([128, 4, 512], bf16, tag=f"xt{sb % 2}{h}",
                                 name=f"xTb{sb}{h}") for h in range(2)]
                xengs = [nc.scalar, nc.gpsimd]
                units = []
                for mc in range(MC):
                    def u(mc=mc, xqTb=xqTb, xTb=xTb):
                        xengs[mc % 2].dma_start(
                            xTb[mc % 2][:, mc // 2, :],
                            xT_d.ap()[mc * 128:(mc + 1) * 128,
                                      sb * 512:(sb + 1) * 512])
                        nc.sync.dma_start(
                            xqTb[:, mc, :],
                            posT_d.ap()[mc * 128:(mc + 1) * 128,
                                        sb * 512:(sb + 1) * 512])
                        nc.vector.tensor_add(xqTb[:, mc, :], xqTb[:, mc, :],
                                             xTb[mc % 2][:, mc // 2, :])
                    units.append(u)
                return (xqTb, xTb), units

            def qk_proj_units(sb, blks, wT, dstT):
                xqTb, _ = blks
                units = []
                for c in range(NC_CH):
                    hold = {}
                    def mk(m0, m1, c=c, hold=hold):
                        def u():
                            if m0 == 0:
                                hold["ps"] = mmp.tile([128, 512], f32,
                                                      tag="mm", name="ps_qk")
                            ps = hold["ps"]
                            for mc in range(m0, m1):
                                nc.tensor.matmul(
                                    ps[:],
                                    wT[mc % 2][:, mc // 2,
                                               c * 128:(c + 1) * 128],
                                    xqTb[:, mc, :],
                                    start=(mc == 0), stop=(mc == MC - 1))
                            if m1 == MC:
                                nc.vector.tensor_copy(
                                    dstT[:, c, sb * 512:(sb + 1) * 512],
                                    ps[:])
                        return u
                    units += [mk(0, 2), mk(2, 4), mk(4, 6), mk(6, 8)]
                return units

            def v_proj_units(sb, blks):
                _, xTb = blks
                units = []
                for stl in range(4):
                    st = sb * 4 + stl
                    hold = {}
                    def uA(stl=stl, hold=hold):
                        ps = mmp.tile([128, 512], f32, tag="mm",
                                      name="ps_v")
                        hold["ps"] = ps
                        for mc in range(4):
                            nc.tensor.matmul(
                                ps[:],
                                xTb[mc % 2][:, mc // 2,
                                            stl * 128:(stl + 1) * 128],
                                wvT[mc % 2][:, mc // 2, :],
                                start=(mc == 0), stop=False)
                    def uB(st=st, stl=stl, hold=hold):
                        ps = hold["ps"]
                        for mc in range(4, MC):
                            nc.tensor.matmul(
                                ps[:],
                                xTb[mc % 2][:, mc // 2,
                                            stl * 128:(stl + 1) * 128],
                                wvT[mc % 2][:, mc // 2, :],
                                start=False, stop=(mc == MC - 1))
                        # heads land even -> cols 0:64, odd -> cols 112:176
                        # of their CW-chunk (two copies: strides differ)
                        vv = V[:, st, :].rearrange("p (c w) -> p c w",
                                                   c=NC_CH)
                        pp = ps[:].rearrange("p (c g h) -> p c g h",
                                             c=NC_CH, g=2)
                        nc.vector.tensor_copy(vv[:, :, 0:64], pp[:, :, 0])
                        nc.vector.tensor_copy(vv[:, :, 112:176],
                                              pp[:, :, 1])
                    units += [uA, uB]
                return units

            def b_proj_units(sb, blks):
                return (qk_proj_units(sb, blks, wqT, QT)
                        + qk_proj_units(sb, blks, wkT, KT)
                        + v_proj_units(sb, blks))

            def d_units(qb):
                units = []
                for qtl in range(4):
                    qt = qb * 4 + qtl
                    osb = outsb.tile([128, DM], f32, tag="osb",
                                     name=f"osb{qt}")
                    for mb in range(2):
                        hold = {}
                        def uA(qt=qt, mb=mb, hold=hold):
                            po = mmp.tile([128, 512], f32, tag="mm",
                                          name="po")
                            hold["po"] = po
                            for c in range(2):
                                nc.tensor.matmul(
                                    po[:],
                                    zTf[:, c, qt * 128:(qt + 1) * 128],
                                    woT[:, c, mb * 512:(mb + 1) * 512],
                                    start=(c == 0), stop=False)
                        def uB(qt=qt, mb=mb, osb=osb, hold=hold):
                            po = hold["po"]
                            for c in range(2, NC_CH):
                                nc.tensor.matmul(
                                    po[:],
                                    zTf[:, c, qt * 128:(qt + 1) * 128],
                                    woT[:, c, mb * 512:(mb + 1) * 512],
                                    start=False, stop=(c == NC_CH - 1))
                            nc.vector.tensor_copy(
                                osb[:, mb * 512:(mb + 1) * 512], po[:])
                            nc.sync.dma_start(
                                out_d.ap()[qt * 128:(qt + 1) * 128,
                                           mb * 512:(mb + 1) * 512],
                                osb[:, mb * 512:(mb + 1) * 512])
                        units += [uA, uB]
                return units

            # D(3) split: chunks 0-2 accumulate mid-wave into d3osb (fp32),
            # so the tail only runs the chunk-3 matmul + add + out DMA.
            d3osb = bigp.tile([128, 4, DM], f32)

            def d3_units():
                a_units, b_units = [], []
                for qtl in range(4):
                    qt = 12 + qtl
                    for mb in range(2):
                        def uA(qt=qt, qtl=qtl, mb=mb):
                            po = mmp.tile([128, 512], f32, tag="mm",
                                          name="po3a")
                            for c in range(3):
                                nc.tensor.matmul(
                                    po[:],
                                    zTf[:, c, qt * 128:(qt + 1) * 128],
                                    woT[:, c, mb * 512:(mb + 1) * 512],
                                    start=(c == 0), stop=(c == 2))
                            nc.vector.tensor_copy(
                                d3osb[:, qtl, mb * 512:(mb + 1) * 512],
                                po[:])
                        def uB(qt=qt, qtl=qtl, mb=mb):
                            po = mmp.tile([128, 512], f32, tag="mm",
                                          name="po3b")
                            nc.tensor.matmul(
                                po[:],
                                zTf[:, 3, qt * 128:(qt + 1) * 128],
                                woT[:, 3, mb * 512:(mb + 1) * 512],
                                start=True, stop=True)
                            nc.vector.tensor_add(
                                d3osb[:, qtl, mb * 512:(mb + 1) * 512],
                                d3osb[:, qtl, mb * 512:(mb + 1) * 512],
                                po[:])
                            nc.scalar.dma_start(
                                out_d.ap()[qt * 128:(qt + 1) * 128,
                                           mb * 512:(mb + 1) * 512],
                                d3osb[:, qtl, mb * 512:(mb + 1) * 512])
                        a_units.append(uA)
                        b_units.append(uB)
                return a_units, b_units

            def emit_c(c, qb, zps):
                nkt = 4 * qb + 4
                for kt in range(nkt):
                    j = kt - 4 * qb
                    diag = j >= 0
                    off = 128 * j if diag else 0
                    sc = scp.tile([128, 2, 512], f32, tag="sc", name="sc")
                    for hh in range(2):
                        r0 = hh * 64
                        nc.tensor.matmul(
                            sc[:, hh, off:512],
                            KT[r0:r0 + 64, c, kt * 128:(kt + 1) * 128],
                            QT[r0:r0 + 64, c,
                               qb * 512 + off:(qb + 1) * 512],
                            start=True, stop=True,
                            tile_position=(r0, 0))
                    ex = expp.tile([128, 2, 512], bf16, tag="ex",
                                   name="ex")
                    nc.scalar.activation(ex[:, :, off:512],
                                         sc[:, :, off:512],
                                         AF.Exp, scale=0.125)
                    if diag:
                        # zero the above-diagonal exp'd entries (DVE, off-PE)
                        nc.vector.tensor_mul(
                            ex[:, :, off:off + 128],
                            ex[:, :, off:off + 128], stair2[:])
                    for hh in range(2):
                        nc.tensor.matmul(
                            zps[:, hh, off:512],
                            V[:, kt, CW * c + 48 * hh:CW * c + 48 * hh + 128],
                            ex[:, hh, off:512],
                            start=(kt == 0), stop=(kt == nkt - 1))
                    yield

            def drain_z(c, zps):
                # single full-tile drain (z_A rows 0:64 + Z_A row 64 in the
                # hh=0 slice; Z_B row 32 + z_B rows 64:128 in hh=1); one op
                # since DVE cost is free-dim-driven, not partition-driven
                nc.vector.tensor_copy(zsb[:, :, c, :], zps[:, :, :])

            def emit_norm(qb, c0, n, dq=None, pe_bcast=False, zps=None):
                dq = dq or nc.sync
                cs = slice(c0, c0 + n)
                # Z rows sit at partitions 64 (head A) / 32 (head B); DMA
                # them to partition 0 (two queues, in parallel) so recip
                # reads from base 0.  On the tail path, read them straight
                # from PSUM so the chain doesn't wait for the full drain.
                zr0 = npl.tile([1, 2, 512], f32, tag="zr0", name="zr0")
                zr1 = npl.tile([1, 2, 512], f32, tag="zr1", name="zr1")
                dq.dma_start(zr0[:, 0:n, :], zsb[64:65, 0, cs, :])
                dq.dma_start(zr1[:, 0:n, :], zsb[32:33, 1, cs, :])
                ri0 = npl.tile([1, 2, 512], f32, tag="ri0", name="ri0")
                ri1 = npl.tile([1, 2, 512], f32, tag="ri1", name="ri1")
                nc.vector.reciprocal_approx_fast(out=ri0[:, 0:n, :],
                                                 in_=zr0[:, 0:n, :])
                nc.vector.reciprocal_approx_fast(out=ri1[:, 0:n, :],
                                                 in_=zr1[:, 0:n, :])
                if pe_bcast:
                    # PE is idle at the tail: broadcast 1/Z across the 64
                    # partitions with two tiny rank-1 matmuls instead of
                    # gpsimd broadcast + partition-shift DMA
                    assert n == 1
                    # bridge 1/Z to bf16 on the (tail-idle) scalar engine
                    rib = npl.tile([1, 2, 512], bf16, tag="rib", name="rib")
                    nc.scalar.copy(rib[:, 0, :], ri0[:, 0, :])
                    nc.scalar.copy(rib[:, 1, :], ri1[:, 0, :])
                    bcp = scp.tile([128, 2, 512], f32, tag="sc", name="bcp")
                    nc.tensor.matmul(bcp[0:64, 0, :], ones_row[:],
                                     rib[0:1, 0, :], start=True, stop=True)
                    nc.tensor.matmul(bcp[64:128, 1, :], ones_row[:],
                                     rib[0:1, 1, :], start=True, stop=True,
                                     tile_position=(0, 64))
                    nc.vector.tensor_mul(
                        zTf[0:64, cs, qb * 512:(qb + 1) * 512],
                        zsb[0:64, 0, cs, :], bcp[0:64, 0:1, :])
                    nc.vector.tensor_mul(
                        zTf[64:128, cs, qb * 512:(qb + 1) * 512],
                        zsb[64:128, 1, cs, :], bcp[64:128, 1:2, :])
                    return
                bcA = npl.tile([64, 2, 512], f32, tag="bcA", name="bcA")
                bcB = npl.tile([128, 2, 512], f32, tag="bcB", name="bcB")
                nc.gpsimd.partition_broadcast(bcA[:, 0:n, :],
                                              ri0[:, 0:n, :], channels=64)
                # stage the B broadcast in bcB's (unused) lower half, then
                # partition-shift it up to 64:128 by DMA
                nc.gpsimd.partition_broadcast(bcB[0:64, 0:n, :],
                                              ri1[:, 0:n, :], channels=64)
                dq.dma_start(bcB[64:128, 0:n, :], bcB[0:64, 0:n, :])
                nc.vector.tensor_mul(
                    zTf[0:64, cs, qb * 512:(qb + 1) * 512],
                    zsb[0:64, 0, cs, :], bcA[:, 0:n, :])
                nc.vector.tensor_mul(
                    zTf[64:128, cs, qb * 512:(qb + 1) * 512],
                    zsb[64:128, 1, cs, :], bcB[64:128, 0:n, :])

            # ---------------- main schedule ---------------------------
            def pipe2(units):
                """Reorder [uA0,uB0,uA1,uB1,...] -> 2-deep software
                pipeline [uA0,uA1,uB0,uA2,uB1,...] so a uB stalled on its
                second DMA half doesn't head-of-line-block a ready uA
                (the PE executes its queue in order; mmp has 2 buffers).
                """
                ua, ub = units[0::2], units[1::2]
                out = [ua[0]]
                for i in range(1, len(ua)):
                    out += [ua[i], ub[i - 1]]
                out.append(ub[-1])
                return out

            blks = {}
            blks[0], lu0 = b_load_units(0)
            wq_u = w_chunk_units(wqT_d, wqT, [nc.sync, nc.scalar])
            for a, b_ in zip(wq_u, lu0):
                a()
                b_()
            for u in w_chunk_units(wkT_d, wkT, [nc.sync, nc.scalar]):
                u()
            for u in pipe2(qk_proj_units(0, blks[0], wqT, QT)):
                u()
            for u in w_chunk_units(wvT_d, wvT, [nc.gpsimd, nc.gpsimd]):
                u()
            for u in pipe2(qk_proj_units(0, blks[0], wkT, KT)):
                u()
            for u in pipe2(v_proj_units(0, blks[0])):
                u()

            # w0: wo+B(1); w1: B(2)+D(0); w2: B(3); w3: D(1)+D(2) through
            # chunks 0-2 then D(3)-first-parts through chunk 3; tail: only
            # the chunk-3 D(3) partials.
            for qb in range(NQB):
                units = []
                if qb == 0:
                    units += wo_units()
                if qb + 1 < NQB:
                    blks[qb + 1], lu = b_load_units(qb + 1)
                    units += lu
                    units += b_proj_units(qb + 1, blks[qb + 1])
                hold_back = []
                if qb == 3:
                    # D(0)/D(1) land in wave 3's ACT-bound slack (the other
                    # waves are PE-oversubscribed by their pinned B-proj
                    # work); D(2) is held back for the tail, keeping the PE
                    # warm while the final norm chain resolves
                    units += d_units(0) + d_units(1)
                    hold_back = d_units(2)
                nkt = 4 * qb + 4
                last = qb == NQB - 1
                # last wave: pace `units` through chunks 0-2 (per-chunk
                # norms), d3a through chunk 3; other waves: pace across all
                # four chunks with norms per chunk pair
                pace_kts = (3 if last else NC_CH) * nkt
                done = 0
                emitted = 0
                for c in range(NC_CH):
                    in_d3 = last and c == NC_CH - 1
                    if in_d3:
                        cur, emitted, pace_kts, done = d3a, 0, nkt, 0
                    else:
                        cur = units
                    zps = zpp.tile([128, 2, 512], f32, tag="z", name="zps")
                    kt_in_c = 0
                    for _ in emit_c(c, qb, zps):
                        done += 1
                        kt_in_c += 1
                        if kt_in_c >= nkt:
                            break  # drain first; catch up after
                        target = min(len(cur),
                                     (len(cur) * done * 4) // (3 * pace_kts))
                        while emitted < target:
                            cur[emitted]()
                            emitted += 1
                    drain_z(c, zps)
                    if last:
                        emit_norm(qb, c, 1,
                                  dq=nc.scalar if c == NC_CH - 1 else None,
                                  pe_bcast=(c == NC_CH - 1), zps=zps)
                        if c == NC_CH - 2:
                            while emitted < len(units):
                                units[emitted]()
                                emitted += 1
                            d3a, d3b = d3_units()
                    elif c % 2 == 1:
                        emit_norm(qb, 2 * (c // 2), 2)
                    target = min(len(cur),
                                 (len(cur) * done * 4) // (3 * pace_kts))
                    while emitted < target:
                        cur[emitted]()
                        emitted += 1
                while emitted < len(cur):
                    cur[emitted]()
                    emitted += 1
            for u in hold_back:
                u()
            for u in d3b:
                u()

    nc.compile()
    return nc


def _get_nc():
    global _BUILT
    if _BUILT is None:
        _BUILT = _build()
    return _BUILT


def _prep_core(x_b, pos_b, wq_g, wk_g, wv_g, wo_g):
    bf = ml_dtypes.bfloat16
    woT = np.empty((128, NC_CH, DM), dtype=np.float32)
    for c in range(NC_CH):
        for hh in range(2):
            woT[hh * 64:(hh + 1) * 64, c, :] = wo_g[2 * c + hh].T
    return {
        "xT_s": np.ascontiguousarray(x_b.T).astype(bf),
        "posT_s": np.ascontiguousarray(pos_b.T).astype(bf),
        "wqT_s": np.ascontiguousarray(wq_g.reshape(IH, DM).T).astype(bf),
        "wkT_s": np.ascontiguousarray(wk_g.reshape(IH, DM).T).astype(bf),
        "wvT_s": np.ascontiguousarray(wv_g.reshape(IH, DM).T).astype(bf),
        "woT_s": woT.astype(bf),
    }


def run(inputs, trace=False):
    from concourse import bass_utils

    nc = _get_nc()
    x = np.asarray(inputs["x"], dtype=np.float32)
    pos = np.asarray(inputs["pos_embed"], dtype=np.float32)
    wq, wk, wv, wo = (np.asarray(inputs[k], dtype=np.float32)
                      for k in ("W_Q", "W_K", "W_V", "W_O"))
    in_maps = []
    for core in range(8):
        b, g = core // 2, core % 2
        hs = slice(g * NH, (g + 1) * NH)
        in_maps.append(_prep_core(x[b], pos[b], wq[hs], wk[hs], wv[hs],
                                  wo[hs]))
    res = bass_utils.run_bass_kernel_spmd(
        nc, in_maps, core_ids=list(range(8)), trace=trace)
    out = np.empty((4, SEQ, DM), dtype=np.float32)
    for b in range(4):
        out[b] = res.results[2 * b]["out_s"] + res.results[2 * b + 1]["out_s"]
    return out, res.exec_time_ns


def kernel(**inputs):
    try:
        out, _ = run(inputs, trace=False)
    except Exception:
        # transient NRT/device errors have been observed on this part; one
        # retry on the already-compiled module is cheap insurance
        out, _ = run(inputs, trace=False)
    return out
